# revision 13
# baseline (speedup 1.0000x reference)
"""Trainium2 Bass kernel for MixExpertAttentionSACNetwork.

Data-parallel over batch: B=4096 split as 512 rows per core across 8 cores,
parameters replicated. Per-core dataflow:

  repT1 = relu(W1p.T @ xT + b1)          feat-on-partition [1024, 512b]
  repT2 = W2.T @ repT1 + b2              feat-on-partition [1024, 512b]
  keys pass (per expert e):
    hkT  = relu(kW1[e].T @ repT2 + kb1)  [512, 512b]
    keys = hkT.T-blocks @ kW2[e]         batch-on-partition [128b, 256] in PSUM
    scores[:, e] = rowsum(keys * q)      (DVE scalar_tensor_tensor, reads PSUM)
  scores += q @ kb2.T (host-folded offset); softmax; expert loss
  vals pass (per expert e):
    hvT, vals as above (+vb2 via ones-matmul); tower_in += w[:, e] * vals
  tower: transpose tower_in -> 3-layer MLP -> q [512]

Matmul precision modes:
  f16x3: operands split into fp16 hi+lo on host (weights) / device
         (activations); each product = 3 fp16 matmuls (hh, hl, lh) at
         1 cyc/row -> 3 cyc/row total, measured MORE accurate than native
         f32 matmul (which costs 4 cyc/row).
  f32:   native f32 matmuls.
  f32r:  reduced-precision 1 cyc/row mode (fast, ~1e-4 rel err).
"""
import sys
sys.path.insert(0, "/opt/trn_rl_repo")

import numpy as np
import concourse.bass as bass
import concourse.mybir as mybir
import concourse.tile as tile
from concourse.bass_utils import run_bass_kernel_spmd

f32 = mybir.dt.float32
f16 = mybir.dt.float16
AF = mybir.ActivationFunctionType
ALU = mybir.AluOpType
AX = mybir.AxisListType

B, OBS, ACT, NT, E, H1, H2, D = 4096, 390, 20, 10, 8, 1024, 512, 256
NCORES = 8
BS = B // NCORES            # 512 rows per core
NBT = BS // 128             # 4 batch tiles
XF = 512                    # padded input feature dim (410 -> 512)
MK1 = H1 // 128             # 8
KX = XF // 128              # 4
MH = H2 // 128              # 4

MODE = "f16x3"              # "f16x3" | "f32" | "f32r"


def split_multi_waits(nc, max_waits=1):
    """This walrus build accepts at most one semaphore wait per instruction;
    hoist extra waits onto single-wait NoOps on the same engine."""
    total = 0
    for fn in nc.m.functions:
        for blk in fn.blocks:
            new_insts = []
            for ins in blk.instructions:
                si = ins.sync_info
                waits = list(si.on_wait) if si and si.on_wait else []
                if len(waits) > max_waits:
                    total += 1
                    keep = waits[-max_waits:]
                    for i, w in enumerate(waits[:-max_waits]):
                        nop = mybir.InstNoOp(name=f"{ins.name}_w{i}", ins=[], outs=[])
                        nop.engine = ins.engine
                        nop.sync_info = mybir.SyncInfo(on_wait=[w], on_update=[])
                        new_insts.append(nop)
                    ins.sync_info = mybir.SyncInfo(
                        on_wait=keep, on_update=list(si.on_update or [])
                    )
                new_insts.append(ins)
            blk.instructions[:] = new_insts
    return total


def build_nc(mode=MODE, reps=1, trace_sim=False):
    x3 = mode == "f16x3"
    nc = bass.Bass(trn_type="TRN2")
    wdt = f16 if x3 else f32   # dram dtype for matmul weights

    def din(name, shape, dt=f32):
        return nc.dram_tensor(name, shape, dt, kind="ExternalInput")

    xT = din("xTh", [XF, BS], wdt)
    xTl = din("xTl", [XF, BS], wdt) if x3 else None
    qh = din("qh", [BS, D])
    qkb = din("qkb", [BS, E])            # host-folded q @ kb2.T score offset
    w1 = din("w1h", [XF, H1], wdt)
    w1l = din("w1l", [XF, H1], wdt) if x3 else None
    w2 = din("w2h", [H1, H1], wdt)
    w2l = din("w2l", [H1, H1], wdt) if x3 else None
    kw1 = din("kw1h", [E * H1, H2], wdt)
    kw1l = din("kw1l", [E * H1, H2], wdt) if x3 else None
    vw1 = din("vw1h", [E * H1, H2], wdt)
    vw1l = din("vw1l", [E * H1, H2], wdt) if x3 else None
    kw2 = din("kw2h", [E * H2, D], wdt)
    kw2l = din("kw2l", [E * H2, D], wdt) if x3 else None
    vw2 = din("vw2h", [E * H2, D], wdt)
    vw2l = din("vw2l", [E * H2, D], wdt) if x3 else None
    b1h = din("b1h", [128, MK1])
    b2h = din("b2h", [128, MK1])
    kb1h = din("kb1h", [128, E * MH])
    vb1h = din("vb1h", [128, E * MH])
    vb2f = din("vb2f", [1, E * D])
    tw1h = din("tw1h", [128, 2 * D])
    tw2h = din("tw2h", [128, 2 * D])
    tw3h = din("tw3h", [128, 2])
    tb1h = din("tb1h", [128, 2])
    tb2h = din("tb2h", [128, 2])
    tb3h = din("tb3h", [1, 1])
    onesd = din("onesd", [128, 128])
    identd = din("identd", [128, 128])

    out_q = nc.dram_tensor("out_q", [1, BS], f32, kind="ExternalOutput")
    out_loss = nc.dram_tensor("out_loss", [1, 1], f32, kind="ExternalOutput")

    use_f32r = mode == "f32r"
    f32r = mybir.dt.float32r

    def MM(out, lhsT, rhs, start, stop):
        if use_f32r:
            lhsT, rhs = lhsT.bitcast(f32r), rhs.bitcast(f32r)
        nc.tensor.matmul(out, lhsT, rhs, start=start, stop=stop)

    def mm3(out, lhs, rhs, first, last):
        """lhs/rhs: (hi, lo) AP pairs in f16x3 mode, plain APs otherwise."""
        if not x3:
            MM(out, lhs, rhs, first, last)
            return
        lh, ll = lhs
        rh, rl = rhs
        nc.tensor.matmul(out, lh, rh, start=first, stop=False)
        nc.tensor.matmul(out, lh, rl, start=False, stop=False)
        nc.tensor.matmul(out, ll, rh, start=False, stop=last)

    with tile.TileContext(nc, trace_sim=trace_sim) as tc:
        with (
            tc.tile_pool(name="const", bufs=1) as cp,
            tc.tile_pool(name="xq", bufs=1) as xp,
            tc.tile_pool(name="wk", bufs=8) as wp,
            tc.tile_pool(name="wk2", bufs=8) as wp2,
            tc.tile_pool(name="act", bufs=1) as ap,
            tc.tile_pool(name="fsc", bufs=3) as fp,
            tc.tile_pool(name="hkv", bufs=2) as hp,
            tc.tile_pool(name="small", bufs=1) as sp,
            tc.tile_pool(name="scr", bufs=4) as scp,
            tc.tile_pool(name="ps", bufs=8, space="PSUM") as ps,
        ):
            # ---- constants
            ones_t = cp.tile([128, 128], f32, tag="ones")
            nc.gpsimd.dma_start(ones_t[:], onesd[:])
            ident_t = cp.tile([128, 128], f32, tag="ident")
            nc.gpsimd.dma_start(ident_t[:], identd[:])
            b1t = cp.tile([128, MK1], f32, tag="b1")
            nc.gpsimd.dma_start(b1t[:], b1h[:])
            b2t = cp.tile([128, MK1], f32, tag="b2")
            nc.gpsimd.dma_start(b2t[:], b2h[:])
            kb1t = cp.tile([128, E * MH], f32, tag="kb1")
            nc.gpsimd.dma_start(kb1t[:], kb1h[:])
            vb1t = cp.tile([128, E * MH], f32, tag="vb1")
            nc.gpsimd.dma_start(vb1t[:], vb1h[:])
            vb2t = cp.tile([1, E * D], f32, tag="vb2")
            nc.gpsimd.dma_start(vb2t[:], vb2f[:])
            tw1t = cp.tile([128, 2 * D], f32, tag="tw1")
            nc.gpsimd.dma_start(tw1t[:], tw1h[:])
            tw2t = cp.tile([128, 2 * D], f32, tag="tw2")
            nc.gpsimd.dma_start(tw2t[:], tw2h[:])
            tw3t = cp.tile([128, 2], f32, tag="tw3")
            nc.gpsimd.dma_start(tw3t[:], tw3h[:])
            tb1t = cp.tile([128, 2], f32, tag="tb1")
            nc.gpsimd.dma_start(tb1t[:], tb1h[:])
            tb2t = cp.tile([128, 2], f32, tag="tb2")
            nc.gpsimd.dma_start(tb2t[:], tb2h[:])
            tb3t = cp.tile([1, 1], f32, tag="tb3")
            nc.gpsimd.dma_start(tb3t[:], tb3h[:])
            eps10 = cp.tile([128, 1], f32, tag="eps10")
            nc.vector.memset(eps10[:], 1e-10)
            ones16 = cp.tile([1, 128], f16, tag="ones16")
            nc.vector.memset(ones16[:], 1.0)
            vb2h16 = cp.tile([1, E * D], f16, tag="vb2h16")
            nc.scalar.copy(vb2h16[:], vb2t[:])
            vb2l16 = cp.tile([1, E * D], f16, tag="vb2l16")
            nc.vector.tensor_sub(vb2l16[:], vb2t[:], vb2h16[:])

            # tower weight splits are invocation-invariant: do once
            if x3:
                tw1sh = cp.tile([128, 2 * D], f16, tag="tw1sh")
                nc.scalar.copy(tw1sh[:], tw1t[:])
                tw1sl = cp.tile([128, 2 * D], f16, tag="tw1sl")
                nc.vector.tensor_sub(tw1sl[:], tw1t[:], tw1sh[:])
                tw2sh = cp.tile([128, 2 * D], f16, tag="tw2sh")
                nc.scalar.copy(tw2sh[:], tw2t[:])
                tw2sl = cp.tile([128, 2 * D], f16, tag="tw2sl")
                nc.vector.tensor_sub(tw2sl[:], tw2t[:], tw2sh[:])
                tw1s = (tw1sh[:], tw1sl[:])
                tw2s = (tw2sh[:], tw2sl[:])
            else:
                tw1s, tw2s = tw1t[:], tw2t[:]

            def emit_body(rep):
                # -- per-invocation inputs
                xt = []
                for k in range(KX):
                    th = xp.tile([128, BS], wdt, tag=f"xt{k}", name=f"xt{k}")
                    nc.sync.dma_start(th[:], xT[k * 128:(k + 1) * 128, :])
                    if x3:
                        tl = xp.tile([128, BS], wdt, tag=f"xtl{k}", name=f"xtl{k}")
                        nc.sync.dma_start(tl[:], xTl[k * 128:(k + 1) * 128, :])
                        xt.append((th[:], tl[:]))
                    else:
                        xt.append(th[:])
                qt, qkbt = [], []
                for bt in range(NBT):
                    t = xp.tile([128, D], f32, tag=f"qt{bt}", name=f"qt{bt}")
                    nc.gpsimd.dma_start(t[:], qh[bt * 128:(bt + 1) * 128, :])
                    qt.append(t)
                    t2 = xp.tile([128, E], f32, tag=f"qkb{bt}", name=f"qkb{bt}")
                    nc.gpsimd.dma_start(t2[:], qkb[bt * 128:(bt + 1) * 128, :])
                    qkbt.append(t2)

                def load_w(dram_h, dram_l, r0, r1, ncols, pool, tag, name):
                    th = pool.tile([128, ncols], wdt, tag=tag, name=name)
                    nc.sync.dma_start(th[:], dram_h[r0:r1, :])
                    if not x3:
                        return th[:]
                    tl = pool.tile([128, ncols], wdt, tag=tag + "l", name=name + "l")
                    nc.sync.dma_start(tl[:], dram_l[r0:r1, :])
                    return (th[:], tl[:])

                def wslice(w, c0, c1):
                    if x3:
                        return (w[0][:, c0:c1], w[1][:, c0:c1])
                    return w[:, c0:c1]

                def split_act(src_f32, pool, tagbase, name, bufs=None):
                    """f32 SBUF tile -> (hi, lo) fp16 pair in f16x3 mode."""
                    if not x3:
                        return src_f32
                    n = src_f32.shape[-1]
                    hi = pool.tile([128, n], f16, tag=tagbase + "h", name=name + "h", bufs=bufs)
                    nc.scalar.copy(hi[:], src_f32)
                    lo = pool.tile([128, n], f16, tag=tagbase + "l", name=name + "l", bufs=bufs)
                    nc.vector.tensor_sub(lo[:], src_f32, hi[:])
                    return (hi[:], lo[:])

                # -- repT1
                ps_r1 = [ps.tile([128, BS], f32, tag="mm", name=f"psr1_{m}") for m in range(MK1)]
                for k in range(KX):
                    w1k = load_w(w1, w1l, k * 128, (k + 1) * 128, H1, wp, "wk", f"w1k{k}")
                    for m in range(MK1):
                        mm3(ps_r1[m][:], wslice(w1k, m * 128, (m + 1) * 128), xt[k],
                            first=(k == 0), last=(k == KX - 1))
                rep1 = []
                for m in range(MK1):
                    t = fp.tile([128, BS], f32, tag="r1f", name=f"rep1_{m}", bufs=(3 if x3 else MK1 + 1))
                    nc.scalar.activation(t[:], ps_r1[m][:], AF.Relu, bias=b1t[:, m:m + 1], scale=1.0)
                    rep1.append(split_act(t[:], hp, "r1s", f"rep1s_{m}", bufs=MK1))

                # -- repT2 (splits persist through both expert passes -> act pool)
                ps_r2 = [ps.tile([128, BS], f32, tag="mm", name=f"psr2_{m}") for m in range(MK1)]
                for k in range(MK1):
                    w2k = load_w(w2, w2l, k * 128, (k + 1) * 128, H1, wp, "wk", f"w2k{k}")
                    for m in range(MK1):
                        mm3(ps_r2[m][:], wslice(w2k, m * 128, (m + 1) * 128), rep1[k],
                            first=(k == 0), last=(k == MK1 - 1))
                rep2 = []
                for m in range(MK1):
                    t = fp.tile([128, BS], f32, tag="r2f", name=f"rep2_{m}", bufs=(3 if x3 else MK1 + 1))
                    nc.scalar.activation(t[:], ps_r2[m][:], AF.Identity, bias=b2t[:, m:m + 1], scale=1.0)
                    if x3:
                        hi = ap.tile([128, BS], f16, tag=f"r2h{m}", name=f"rep2h_{m}")
                        nc.scalar.copy(hi[:], t[:])
                        lo = ap.tile([128, BS], f16, tag=f"r2l{m}", name=f"rep2l_{m}")
                        nc.vector.tensor_sub(lo[:], t[:], hi[:])
                        rep2.append((hi[:], lo[:]))
                    else:
                        rep2.append(t[:])

                scores = [sp.tile([128, E], f32, tag=f"sc{bt}", name=f"scores{bt}") for bt in range(NBT)]

                def expert_pass(e, ew1, ew1l, ew2, ew2l, eb1t, kind, final_stop=True):
                    ps_h = [ps.tile([128, BS], f32, tag="mm", name=f"ps{kind}h{e}_{m}") for m in range(MH)]
                    for k in range(MK1):
                        wkt = load_w(ew1, ew1l, e * H1 + k * 128, e * H1 + (k + 1) * 128,
                                     H2, wp, "wk", f"{kind}w1_{e}_{k}")
                        for m in range(MH):
                            mm3(ps_h[m][:], wslice(wkt, m * 128, (m + 1) * 128), rep2[k],
                                first=(k == 0), last=(k == MK1 - 1))
                    ht = []
                    for m in range(MH):
                        t = fp.tile([128, BS], f32, tag="hf", name=f"h{kind}{e}_{m}", bufs=(3 if x3 else 2 * MH))
                        nc.scalar.activation(t[:], ps_h[m][:], AF.Relu,
                                             bias=eb1t[:, e * MH + m:e * MH + m + 1], scale=1.0)
                        ht.append(split_act(t[:], hp, f"h{kind}{m}", f"h{kind}s{e}_{m}"))
                    w2_tiles = [
                        load_w(ew2, ew2l, e * H2 + m * 128, e * H2 + (m + 1) * 128,
                               D, wp2, "kv", f"{kind}w2_{e}_{m}")
                        for m in range(MH)
                    ]
                    ps_o = []
                    for bt in range(NBT):
                        po = ps.tile([128, D], f32, tag="mm", name=f"ps{kind}o{e}_{bt}")
                        for m in range(MH):
                            if x3:
                                hh, hl = ht[m]
                                lhs = (hh[:, bt * 128:(bt + 1) * 128], hl[:, bt * 128:(bt + 1) * 128])
                            else:
                                lhs = ht[m][:, bt * 128:(bt + 1) * 128]
                            mm3(po[:], lhs, w2_tiles[m], first=(m == 0),
                                last=(final_stop and m == MH - 1))
                        ps_o.append(po)
                    return ps_o

                # -- keys pass
                for e in range(E):
                    ps_o = expert_pass(e, kw1, kw1l, kw2, kw2l, kb1t, "k")
                    for bt in range(NBT):
                        scr = scp.tile([128, D], f32, tag="scr", name=f"scrk{e}_{bt}")
                        nc.vector.scalar_tensor_tensor(
                            scr[:], ps_o[bt][:], 1.0, qt[bt][:],
                            op0=ALU.mult, op1=ALU.mult,
                            accum_out=scores[bt][:, e:e + 1],
                        )

                # -- softmax + loss pieces (fold in host-computed q@kb2.T first)
                wts = []
                lossv = sp.tile([128, NBT], f32, tag="lossv", name="lossv")
                for bt in range(NBT):
                    nc.vector.tensor_add(scores[bt][:], scores[bt][:], qkbt[bt][:])
                    negmax = sp.tile([128, 1], f32, tag=f"nm{bt}", name=f"negmax{bt}")
                    nc.vector.tensor_reduce(negmax[:], scores[bt][:], axis=AX.X, op=ALU.max, negate=True)
                    expw = sp.tile([128, E], f32, tag=f"ew{bt}", name=f"expw{bt}")
                    nc.scalar.activation(expw[:], scores[bt][:], AF.Exp, bias=negmax[:, 0:1], scale=1.0)
                    ssum = sp.tile([128, 1], f32, tag=f"ss{bt}", name=f"ssum{bt}")
                    nc.vector.tensor_reduce(ssum[:], expw[:], axis=AX.X, op=ALU.add)
                    rinv = sp.tile([128, 1], f32, tag=f"ri{bt}", name=f"rinv{bt}")
                    nc.vector.reciprocal(rinv[:], ssum[:])
                    wt = sp.tile([128, E], f32, tag=f"wt{bt}", name=f"wt{bt}")
                    nc.vector.tensor_scalar_mul(wt[:], expw[:], rinv[:, 0:1])
                    wts.append(wt)
                    logw = sp.tile([128, E], f32, tag=f"lw{bt}", name=f"logw{bt}")
                    nc.scalar.activation(logw[:], wt[:], AF.Ln, bias=eps10[:, 0:1], scale=1.0)
                    clipw = sp.tile([128, E], f32, tag=f"cw{bt}", name=f"clipw{bt}")
                    nc.vector.tensor_scalar(clipw[:], logw[:], -6.0, 0.0, op0=ALU.max, op1=ALU.min)
                    nc.vector.tensor_reduce(lossv[:, bt:bt + 1], clipw[:], axis=AX.X, op=ALU.add)

                # -- vals pass (vb2 bias via ones-matmul, weighted accumulate)
                acc = [ap.tile([128, D], f32, tag=f"acc{bt}", name=f"acc{bt}") for bt in range(NBT)]
                for e in range(E):
                    ps_o = expert_pass(e, vw1, vw1l, vw2, vw2l, vb1t, "v", final_stop=False)
                    for bt in range(NBT):
                        nc.tensor.matmul(ps_o[bt][:], ones16[0:1, :], vb2h16[0:1, e * D:(e + 1) * D],
                                         start=False, stop=False, skip_group_check=True)
                        nc.tensor.matmul(ps_o[bt][:], ones16[0:1, :], vb2l16[0:1, e * D:(e + 1) * D],
                                         start=False, stop=True, skip_group_check=True)
                        if e == 0:
                            nc.vector.tensor_scalar_mul(acc[bt][:], ps_o[bt][:], wts[bt][:, 0:1])
                        else:
                            nc.vector.scalar_tensor_tensor(
                                acc[bt][:], ps_o[bt][:], wts[bt][:, e:e + 1], acc[bt][:],
                                op0=ALU.mult, op1=ALU.add,
                            )

                # -- loss reduction
                ps_l = ps.tile([1, NBT], f32, tag="mm", name="ps_loss")
                nc.tensor.matmul(ps_l[:], ones_t[:, 0:1], lossv[:], start=True, stop=True)
                ls_sb = sp.tile([1, 1], f32, tag="lsum", name="ls_sb")
                nc.vector.tensor_reduce(ls_sb[:], ps_l[:], axis=AX.X, op=ALU.add)
                nc.sync.dma_start(out_loss[:], ls_sb[:])

                # -- tower
                tt = []
                for f in range(2):
                    t = sp.tile([128, BS], f32, tag=f"tt{f}", name=f"towerT{f}")
                    tt.append(t)
                for bt in range(NBT):
                    for f in range(2):
                        ptp = ps.tile([128, 128], f32, tag="mm", name=f"pst{bt}_{f}")
                        nc.tensor.transpose(ptp[:], acc[bt][:, f * 128:(f + 1) * 128], ident_t[:])
                        nc.scalar.copy(tt[f][:, bt * 128:(bt + 1) * 128], ptp[:])
                tts = [split_act(tt[f][:], hp, f"tts{f}", f"tts{f}") for f in range(2)]

                def twslice(w, c0, c1):
                    if x3:
                        return (w[0][:, c0:c1], w[1][:, c0:c1])
                    return w[:, c0:c1]

                h1s = []
                for f in range(2):
                    ph = ps.tile([128, BS], f32, tag="mm", name=f"ps_h1_{f}")
                    for k in range(2):
                        mm3(ph[:], twslice(tw1s, k * D + f * 128, k * D + f * 128 + 128), tts[k],
                            first=(k == 0), last=(k == 1))
                    t = fp.tile([128, BS], f32, tag="twf", name=f"h1_{f}", bufs=(3 if x3 else 4))
                    nc.scalar.activation(t[:], ph[:], AF.Relu, bias=tb1t[:, f:f + 1], scale=1.0)
                    h1s.append(split_act(t[:], hp, f"h1s{f}", f"h1s_{f}"))
                h2f = []
                for f in range(2):
                    ph = ps.tile([128, BS], f32, tag="mm", name=f"ps_h2_{f}")
                    for k in range(2):
                        mm3(ph[:], twslice(tw2s, k * D + f * 128, k * D + f * 128 + 128), h1s[k],
                            first=(k == 0), last=(k == 1))
                    t = fp.tile([128, BS], f32, tag="twf2", name=f"h2_{f}")
                    nc.scalar.activation(t[:], ph[:], AF.Relu, bias=tb2t[:, f:f + 1], scale=1.0)
                    h2f.append(t)
                ps_q = ps.tile([1, BS], f32, tag="mm", name="ps_q")
                for k in range(2):
                    nc.tensor.matmul(ps_q[:], tw3t[:, k:k + 1], h2f[k][:], start=(k == 0), stop=(k == 1))
                q_sb = sp.tile([1, BS], f32, tag="qsb", name="q_sb")
                nc.scalar.activation(q_sb[:], ps_q[:], AF.Identity, bias=tb3t[0:1, 0:1], scale=1.0)
                nc.sync.dma_start(out_q[:], q_sb[:])

            for _r in range(reps):
                emit_body(_r)

    split_multi_waits(nc)
    return nc


_NC_CACHE = {}


def _get_nc(mode=MODE):
    if mode not in _NC_CACHE:
        _NC_CACHE[mode] = build_nc(mode=mode)
    return _NC_CACHE[mode]


def prepare_in_maps(state_feat, act, task_id, rep_W1, rep_b1, rep_W2, rep_b2, emb,
                    kW1, kb1, kW2, kb2, vW1, vb1, vW2, vb2,
                    tW1, tb1, tW2, tb2, tW3, tb3, mode=MODE):
    x3 = mode == "f16x3"
    f32c = lambda a: np.ascontiguousarray(np.asarray(a, np.float32))

    x = np.concatenate([np.asarray(state_feat, np.float32),
                        np.asarray(act, np.float32)], axis=1)
    xp_ = np.zeros((B, XF), np.float32)
    xp_[:, :OBS + ACT] = x
    query = np.tanh(np.asarray(emb, np.float32))[np.asarray(task_id)]   # [B, D]
    qkb = query @ f32c(kb2).T                                           # [B, E]

    w1 = np.zeros((XF, H1), np.float32)
    w1[:OBS + ACT, :] = np.asarray(rep_W1, np.float32)
    w2 = f32c(rep_W2)
    kw1 = f32c(kW1).reshape(E * H1, H2)
    vw1 = f32c(vW1).reshape(E * H1, H2)
    kw2 = f32c(kW2).reshape(E * H2, D)
    vw2 = f32c(vW2).reshape(E * H2, D)

    def hl(a):
        hi = a.astype(np.float16)
        lo = (a - hi.astype(np.float32)).astype(np.float16)
        return np.ascontiguousarray(hi), np.ascontiguousarray(lo)

    shared = {}
    for nm, a in [("w1", w1), ("w2", w2), ("kw1", kw1), ("vw1", vw1),
                  ("kw2", kw2), ("vw2", vw2)]:
        if x3:
            shared[nm + "h"], shared[nm + "l"] = hl(a)
        else:
            shared[nm + "h"] = np.ascontiguousarray(a)

    shared.update({
        "b1h": np.ascontiguousarray(f32c(rep_b1).reshape(MK1, 128).T),
        "b2h": np.ascontiguousarray(f32c(rep_b2).reshape(MK1, 128).T),
        "kb1h": np.ascontiguousarray(f32c(kb1).reshape(E, MH, 128).transpose(2, 0, 1).reshape(128, E * MH)),
        "vb1h": np.ascontiguousarray(f32c(vb1).reshape(E, MH, 128).transpose(2, 0, 1).reshape(128, E * MH)),
        "vb2f": f32c(vb2).reshape(1, E * D),
        "tw1h": np.ascontiguousarray(f32c(tW1).reshape(2, 128, D).transpose(1, 0, 2).reshape(128, 2 * D)),
        "tw2h": np.ascontiguousarray(f32c(tW2).reshape(2, 128, D).transpose(1, 0, 2).reshape(128, 2 * D)),
        "tw3h": np.ascontiguousarray(f32c(tW3).reshape(2, 128).T),
        "tb1h": np.ascontiguousarray(f32c(tb1).reshape(2, 128).T),
        "tb2h": np.ascontiguousarray(f32c(tb2).reshape(2, 128).T),
        "tb3h": f32c(tb3).reshape(1, 1),
        "onesd": np.ones((128, 128), np.float32),
        "identd": np.eye(128, dtype=np.float32),
    })

    in_maps = []
    for c in range(NCORES):
        sl = slice(c * BS, (c + 1) * BS)
        m = dict(shared)
        xs = np.ascontiguousarray(xp_[sl].T)
        if x3:
            m["xTh"], m["xTl"] = hl(xs)
        else:
            m["xTh"] = xs
        m["qh"] = np.ascontiguousarray(query[sl])
        m["qkb"] = np.ascontiguousarray(qkb[sl])
        in_maps.append(m)
    return in_maps


def kernel(**inputs):
    in_maps = prepare_in_maps(**inputs)
    nc = _get_nc()
    res = run_bass_kernel_spmd(nc, in_maps, core_ids=list(range(NCORES)))
    q = np.concatenate([res.results[c]["out_q"][0] for c in range(NCORES)])
    total = sum(float(res.results[c]["out_loss"][0, 0]) for c in range(NCORES))
    expert_loss = np.float32(-0.3 * total / B)
    return q.astype(np.float32), expert_loss


# revision 14
# speedup vs baseline: 1.0746x; 1.0746x over previous
"""Trainium2 Bass kernel for MixExpertAttentionSACNetwork.

Data-parallel over batch: B=4096 split as 512 rows per core across 8 cores,
parameters replicated. Per-core dataflow:

  repT1 = relu(W1p.T @ xT + b1)          feat-on-partition [1024, 512b]
  repT2 = W2.T @ repT1 + b2              feat-on-partition [1024, 512b]
  keys pass (per expert e):
    hkT  = relu(kW1[e].T @ repT2 + kb1)  [512, 512b]
    keys = hkT.T-blocks @ kW2[e]         batch-on-partition [128b, 256] in PSUM
    scores[:, e] = rowsum(keys * q)      (DVE scalar_tensor_tensor, reads PSUM)
  scores += q @ kb2.T (host-folded offset); softmax; expert loss
  vals pass (per expert e):
    hvT, vals as above (+vb2 via ones-matmul); tower_in += w[:, e] * vals
  tower: transpose tower_in -> 3-layer MLP -> q [512]

Matmul precision modes:
  f16x3: operands split into fp16 hi+lo on host (weights) / device
         (activations); each product = 3 fp16 matmuls (hh, hl, lh) at
         1 cyc/row -> 3 cyc/row total, measured MORE accurate than native
         f32 matmul (which costs 4 cyc/row).
  f32:   native f32 matmuls.
  f32r:  reduced-precision 1 cyc/row mode (fast, ~1e-4 rel err).
"""
import sys
sys.path.insert(0, "/opt/trn_rl_repo")

import numpy as np
import concourse.bass as bass
import concourse.mybir as mybir
import concourse.tile as tile
from concourse.bass_utils import run_bass_kernel_spmd

f32 = mybir.dt.float32
f16 = mybir.dt.float16
AF = mybir.ActivationFunctionType
ALU = mybir.AluOpType
AX = mybir.AxisListType

B, OBS, ACT, NT, E, H1, H2, D = 4096, 390, 20, 10, 8, 1024, 512, 256
NCORES = 8
BS = B // NCORES            # 512 rows per core
NBT = BS // 128             # 4 batch tiles
XF = 512                    # padded input feature dim (410 -> 512)
MK1 = H1 // 128             # 8
KX = XF // 128              # 4
MH = H2 // 128              # 4

MODE = "f16x3"              # "f16x3" | "f32" | "f32r"


def split_multi_waits(nc, max_waits=1):
    """This walrus build accepts at most one semaphore wait per instruction;
    hoist extra waits onto single-wait NoOps on the same engine."""
    total = 0
    for fn in nc.m.functions:
        for blk in fn.blocks:
            new_insts = []
            for ins in blk.instructions:
                si = ins.sync_info
                waits = list(si.on_wait) if si and si.on_wait else []
                if len(waits) > max_waits:
                    total += 1
                    keep = waits[-max_waits:]
                    for i, w in enumerate(waits[:-max_waits]):
                        nop = mybir.InstNoOp(name=f"{ins.name}_w{i}", ins=[], outs=[])
                        nop.engine = ins.engine
                        nop.sync_info = mybir.SyncInfo(on_wait=[w], on_update=[])
                        new_insts.append(nop)
                    ins.sync_info = mybir.SyncInfo(
                        on_wait=keep, on_update=list(si.on_update or [])
                    )
                new_insts.append(ins)
            blk.instructions[:] = new_insts
    return total


def build_nc(mode=MODE, reps=1, trace_sim=False):
    x3 = mode == "f16x3"
    nc = bass.Bass(trn_type="TRN2")
    wdt = f16 if x3 else f32   # dram dtype for matmul weights

    def din(name, shape, dt=f32):
        return nc.dram_tensor(name, shape, dt, kind="ExternalInput")

    xT = din("xTh", [XF, BS], wdt)
    xTl = din("xTl", [XF, BS], wdt) if x3 else None
    qh = din("qh", [BS, D])
    qkb = din("qkb", [BS, E])            # host-folded q @ kb2.T score offset
    w1 = din("w1h", [XF, H1], wdt)
    w1l = din("w1l", [XF, H1], wdt) if x3 else None
    w2 = din("w2h", [H1, H1], wdt)
    w2l = din("w2l", [H1, H1], wdt) if x3 else None
    kw1 = din("kw1h", [E * H1, H2], wdt)
    kw1l = din("kw1l", [E * H1, H2], wdt) if x3 else None
    vw1 = din("vw1h", [E * H1, H2], wdt)
    vw1l = din("vw1l", [E * H1, H2], wdt) if x3 else None
    kw2 = din("kw2h", [E * H2, D], wdt)
    kw2l = din("kw2l", [E * H2, D], wdt) if x3 else None
    vw2 = din("vw2h", [E * H2, D], wdt)
    vw2l = din("vw2l", [E * H2, D], wdt) if x3 else None
    b1h = din("b1h", [128, MK1])
    b2h = din("b2h", [128, MK1])
    kb1h = din("kb1h", [128, E * MH])
    vb1h = din("vb1h", [128, E * MH])
    vb2f = din("vb2f", [1, E * D])
    tw1h = din("tw1h", [128, 2 * D])
    tw2h = din("tw2h", [128, 2 * D])
    tw3h = din("tw3h", [128, 2])
    tb1h = din("tb1h", [128, 2])
    tb2h = din("tb2h", [128, 2])
    tb3h = din("tb3h", [1, 1])
    onesd = din("onesd", [128, 128])
    identd = din("identd", [128, 128])

    out_q = nc.dram_tensor("out_q", [1, BS], f32, kind="ExternalOutput")
    out_loss = nc.dram_tensor("out_loss", [1, 1], f32, kind="ExternalOutput")

    use_f32r = mode == "f32r"
    f32r = mybir.dt.float32r

    def MM(out, lhsT, rhs, start, stop):
        if use_f32r:
            lhsT, rhs = lhsT.bitcast(f32r), rhs.bitcast(f32r)
        nc.tensor.matmul(out, lhsT, rhs, start=start, stop=stop)

    def mm3(out, lhs, rhs, first, last):
        """lhs/rhs: (hi, lo) AP pairs in f16x3 mode, plain APs otherwise."""
        if not x3:
            MM(out, lhs, rhs, first, last)
            return
        lh, ll = lhs
        rh, rl = rhs
        nc.tensor.matmul(out, lh, rh, start=first, stop=False)
        nc.tensor.matmul(out, lh, rl, start=False, stop=False)
        nc.tensor.matmul(out, ll, rh, start=False, stop=last)

    with tile.TileContext(nc, trace_sim=trace_sim) as tc:
        with (
            tc.tile_pool(name="const", bufs=1) as cp,
            tc.tile_pool(name="xq", bufs=1) as xp,
            tc.tile_pool(name="wk", bufs=8) as wp,
            tc.tile_pool(name="wk2", bufs=8) as wp2,
            tc.tile_pool(name="act", bufs=1) as ap,
            tc.tile_pool(name="fsc", bufs=3) as fp,
            tc.tile_pool(name="hkv", bufs=2) as hp,
            tc.tile_pool(name="small", bufs=1) as sp,
            tc.tile_pool(name="scr", bufs=4) as scp,
            tc.tile_pool(name="ps", bufs=8, space="PSUM") as ps,
        ):
            # ---- constants
            ones_t = cp.tile([128, 128], f32, tag="ones")
            nc.sync.dma_start(ones_t[:], onesd[:])
            ident_t = cp.tile([128, 128], f32, tag="ident")
            nc.sync.dma_start(ident_t[:], identd[:])
            b1t = cp.tile([128, MK1], f32, tag="b1")
            nc.sync.dma_start(b1t[:], b1h[:])
            b2t = cp.tile([128, MK1], f32, tag="b2")
            nc.sync.dma_start(b2t[:], b2h[:])
            kb1t = cp.tile([128, E * MH], f32, tag="kb1")
            nc.sync.dma_start(kb1t[:], kb1h[:])
            vb1t = cp.tile([128, E * MH], f32, tag="vb1")
            nc.sync.dma_start(vb1t[:], vb1h[:])
            vb2t = cp.tile([1, E * D], f32, tag="vb2")
            nc.sync.dma_start(vb2t[:], vb2f[:])
            tw1t = cp.tile([128, 2 * D], f32, tag="tw1")
            nc.sync.dma_start(tw1t[:], tw1h[:])
            tw2t = cp.tile([128, 2 * D], f32, tag="tw2")
            nc.sync.dma_start(tw2t[:], tw2h[:])
            tw3t = cp.tile([128, 2], f32, tag="tw3")
            nc.sync.dma_start(tw3t[:], tw3h[:])
            tb1t = cp.tile([128, 2], f32, tag="tb1")
            nc.sync.dma_start(tb1t[:], tb1h[:])
            tb2t = cp.tile([128, 2], f32, tag="tb2")
            nc.sync.dma_start(tb2t[:], tb2h[:])
            tb3t = cp.tile([1, 1], f32, tag="tb3")
            nc.sync.dma_start(tb3t[:], tb3h[:])
            eps10 = cp.tile([128, 1], f32, tag="eps10")
            nc.vector.memset(eps10[:], 1e-10)
            ones16 = cp.tile([1, 128], f16, tag="ones16")
            nc.vector.memset(ones16[:], 1.0)
            vb2h16 = cp.tile([1, E * D], f16, tag="vb2h16")
            nc.scalar.copy(vb2h16[:], vb2t[:])
            vb2l16 = cp.tile([1, E * D], f16, tag="vb2l16")
            nc.vector.tensor_sub(vb2l16[:], vb2t[:], vb2h16[:])

            # tower weight splits are invocation-invariant: do once
            if x3:
                tw1sh = cp.tile([128, 2 * D], f16, tag="tw1sh")
                nc.scalar.copy(tw1sh[:], tw1t[:])
                tw1sl = cp.tile([128, 2 * D], f16, tag="tw1sl")
                nc.vector.tensor_sub(tw1sl[:], tw1t[:], tw1sh[:])
                tw2sh = cp.tile([128, 2 * D], f16, tag="tw2sh")
                nc.scalar.copy(tw2sh[:], tw2t[:])
                tw2sl = cp.tile([128, 2 * D], f16, tag="tw2sl")
                nc.vector.tensor_sub(tw2sl[:], tw2t[:], tw2sh[:])
                tw1s = (tw1sh[:], tw1sl[:])
                tw2s = (tw2sh[:], tw2sl[:])
            else:
                tw1s, tw2s = tw1t[:], tw2t[:]

            def emit_body(rep):
                # -- per-invocation inputs
                xt = []
                for k in range(KX):
                    th = xp.tile([128, BS], wdt, tag=f"xt{k}", name=f"xt{k}")
                    nc.sync.dma_start(th[:], xT[k * 128:(k + 1) * 128, :])
                    if x3:
                        tl = xp.tile([128, BS], wdt, tag=f"xtl{k}", name=f"xtl{k}")
                        nc.sync.dma_start(tl[:], xTl[k * 128:(k + 1) * 128, :])
                        xt.append((th[:], tl[:]))
                    else:
                        xt.append(th[:])
                qt, qkbt = [], []
                for bt in range(NBT):
                    t = xp.tile([128, D], f32, tag=f"qt{bt}", name=f"qt{bt}")
                    nc.sync.dma_start(t[:], qh[bt * 128:(bt + 1) * 128, :])
                    qt.append(t)
                    t2 = xp.tile([128, E], f32, tag=f"qkb{bt}", name=f"qkb{bt}")
                    nc.sync.dma_start(t2[:], qkb[bt * 128:(bt + 1) * 128, :])
                    qkbt.append(t2)

                def load_w(dram_h, dram_l, r0, r1, ncols, pool, tag, name):
                    th = pool.tile([128, ncols], wdt, tag=tag, name=name)
                    nc.sync.dma_start(th[:], dram_h[r0:r1, :])
                    if not x3:
                        return th[:]
                    tl = pool.tile([128, ncols], wdt, tag=tag + "l", name=name + "l")
                    nc.sync.dma_start(tl[:], dram_l[r0:r1, :])
                    return (th[:], tl[:])

                def wslice(w, c0, c1):
                    if x3:
                        return (w[0][:, c0:c1], w[1][:, c0:c1])
                    return w[:, c0:c1]

                def split_act(src_f32, pool, tagbase, name, bufs=None):
                    """f32 SBUF tile -> (hi, lo) fp16 pair in f16x3 mode."""
                    if not x3:
                        return src_f32
                    n = src_f32.shape[-1]
                    hi = pool.tile([128, n], f16, tag=tagbase + "h", name=name + "h", bufs=bufs)
                    nc.scalar.copy(hi[:], src_f32)
                    lo = pool.tile([128, n], f16, tag=tagbase + "l", name=name + "l", bufs=bufs)
                    nc.vector.tensor_sub(lo[:], src_f32, hi[:])
                    return (hi[:], lo[:])

                # -- repT1
                ps_r1 = [ps.tile([128, BS], f32, tag="mm", name=f"psr1_{m}") for m in range(MK1)]
                for k in range(KX):
                    w1k = load_w(w1, w1l, k * 128, (k + 1) * 128, H1, wp, "wk", f"w1k{k}")
                    for m in range(MK1):
                        mm3(ps_r1[m][:], wslice(w1k, m * 128, (m + 1) * 128), xt[k],
                            first=(k == 0), last=(k == KX - 1))
                rep1 = []
                for m in range(MK1):
                    t = fp.tile([128, BS], f32, tag="r1f", name=f"rep1_{m}", bufs=(3 if x3 else MK1 + 1))
                    nc.scalar.activation(t[:], ps_r1[m][:], AF.Relu, bias=b1t[:, m:m + 1], scale=1.0)
                    rep1.append(split_act(t[:], hp, "r1s", f"rep1s_{m}", bufs=MK1))

                # -- repT2 (splits persist through both expert passes -> act pool)
                ps_r2 = [ps.tile([128, BS], f32, tag="mm", name=f"psr2_{m}") for m in range(MK1)]
                for k in range(MK1):
                    w2k = load_w(w2, w2l, k * 128, (k + 1) * 128, H1, wp, "wk", f"w2k{k}")
                    for m in range(MK1):
                        mm3(ps_r2[m][:], wslice(w2k, m * 128, (m + 1) * 128), rep1[k],
                            first=(k == 0), last=(k == MK1 - 1))
                rep2 = []
                for m in range(MK1):
                    t = fp.tile([128, BS], f32, tag="r2f", name=f"rep2_{m}", bufs=(3 if x3 else MK1 + 1))
                    nc.scalar.activation(t[:], ps_r2[m][:], AF.Identity, bias=b2t[:, m:m + 1], scale=1.0)
                    if x3:
                        hi = ap.tile([128, BS], f16, tag=f"r2h{m}", name=f"rep2h_{m}")
                        nc.scalar.copy(hi[:], t[:])
                        lo = ap.tile([128, BS], f16, tag=f"r2l{m}", name=f"rep2l_{m}")
                        nc.vector.tensor_sub(lo[:], t[:], hi[:])
                        rep2.append((hi[:], lo[:]))
                    else:
                        rep2.append(t[:])

                scores = [sp.tile([128, E], f32, tag=f"sc{bt}", name=f"scores{bt}") for bt in range(NBT)]

                def expert_pass(e, ew1, ew1l, ew2, ew2l, eb1t, kind, final_stop=True):
                    ps_h = [ps.tile([128, BS], f32, tag="mm", name=f"ps{kind}h{e}_{m}") for m in range(MH)]
                    for k in range(MK1):
                        wkt = load_w(ew1, ew1l, e * H1 + k * 128, e * H1 + (k + 1) * 128,
                                     H2, wp, "wk", f"{kind}w1_{e}_{k}")
                        for m in range(MH):
                            mm3(ps_h[m][:], wslice(wkt, m * 128, (m + 1) * 128), rep2[k],
                                first=(k == 0), last=(k == MK1 - 1))
                    ht = []
                    for m in range(MH):
                        t = fp.tile([128, BS], f32, tag="hf", name=f"h{kind}{e}_{m}", bufs=(3 if x3 else 2 * MH))
                        nc.scalar.activation(t[:], ps_h[m][:], AF.Relu,
                                             bias=eb1t[:, e * MH + m:e * MH + m + 1], scale=1.0)
                        ht.append(split_act(t[:], hp, f"h{kind}{m}", f"h{kind}s{e}_{m}"))
                    w2_tiles = [
                        load_w(ew2, ew2l, e * H2 + m * 128, e * H2 + (m + 1) * 128,
                               D, wp2, "kv", f"{kind}w2_{e}_{m}")
                        for m in range(MH)
                    ]
                    ps_o = []
                    for bt in range(NBT):
                        po = ps.tile([128, D], f32, tag="mm", name=f"ps{kind}o{e}_{bt}")
                        for m in range(MH):
                            if x3:
                                hh, hl = ht[m]
                                lhs = (hh[:, bt * 128:(bt + 1) * 128], hl[:, bt * 128:(bt + 1) * 128])
                            else:
                                lhs = ht[m][:, bt * 128:(bt + 1) * 128]
                            mm3(po[:], lhs, w2_tiles[m], first=(m == 0),
                                last=(final_stop and m == MH - 1))
                        ps_o.append(po)
                    return ps_o

                # -- keys pass
                for e in range(E):
                    ps_o = expert_pass(e, kw1, kw1l, kw2, kw2l, kb1t, "k")
                    for bt in range(NBT):
                        scr = scp.tile([128, D], f32, tag="scr", name=f"scrk{e}_{bt}")
                        nc.vector.scalar_tensor_tensor(
                            scr[:], ps_o[bt][:], 1.0, qt[bt][:],
                            op0=ALU.mult, op1=ALU.mult,
                            accum_out=scores[bt][:, e:e + 1],
                        )

                # -- softmax + loss pieces (fold in host-computed q@kb2.T first)
                wts = []
                lossv = sp.tile([128, NBT], f32, tag="lossv", name="lossv")
                for bt in range(NBT):
                    nc.vector.tensor_add(scores[bt][:], scores[bt][:], qkbt[bt][:])
                    negmax = sp.tile([128, 1], f32, tag=f"nm{bt}", name=f"negmax{bt}")
                    nc.vector.tensor_reduce(negmax[:], scores[bt][:], axis=AX.X, op=ALU.max, negate=True)
                    expw = sp.tile([128, E], f32, tag=f"ew{bt}", name=f"expw{bt}")
                    nc.scalar.activation(expw[:], scores[bt][:], AF.Exp, bias=negmax[:, 0:1], scale=1.0)
                    ssum = sp.tile([128, 1], f32, tag=f"ss{bt}", name=f"ssum{bt}")
                    nc.vector.tensor_reduce(ssum[:], expw[:], axis=AX.X, op=ALU.add)
                    rinv = sp.tile([128, 1], f32, tag=f"ri{bt}", name=f"rinv{bt}")
                    nc.vector.reciprocal(rinv[:], ssum[:])
                    wt = sp.tile([128, E], f32, tag=f"wt{bt}", name=f"wt{bt}")
                    nc.vector.tensor_scalar_mul(wt[:], expw[:], rinv[:, 0:1])
                    wts.append(wt)
                    logw = sp.tile([128, E], f32, tag=f"lw{bt}", name=f"logw{bt}")
                    nc.scalar.activation(logw[:], wt[:], AF.Ln, bias=eps10[:, 0:1], scale=1.0)
                    clipw = sp.tile([128, E], f32, tag=f"cw{bt}", name=f"clipw{bt}")
                    nc.vector.tensor_scalar(clipw[:], logw[:], -6.0, 0.0, op0=ALU.max, op1=ALU.min)
                    nc.vector.tensor_reduce(lossv[:, bt:bt + 1], clipw[:], axis=AX.X, op=ALU.add)

                # -- vals pass (vb2 bias via ones-matmul, weighted accumulate)
                acc = [ap.tile([128, D], f32, tag=f"acc{bt}", name=f"acc{bt}") for bt in range(NBT)]
                for e in range(E):
                    ps_o = expert_pass(e, vw1, vw1l, vw2, vw2l, vb1t, "v", final_stop=False)
                    for bt in range(NBT):
                        nc.tensor.matmul(ps_o[bt][:], ones16[0:1, :], vb2h16[0:1, e * D:(e + 1) * D],
                                         start=False, stop=False, skip_group_check=True)
                        nc.tensor.matmul(ps_o[bt][:], ones16[0:1, :], vb2l16[0:1, e * D:(e + 1) * D],
                                         start=False, stop=True, skip_group_check=True)
                        if e == 0:
                            nc.vector.tensor_scalar_mul(acc[bt][:], ps_o[bt][:], wts[bt][:, 0:1])
                        else:
                            nc.vector.scalar_tensor_tensor(
                                acc[bt][:], ps_o[bt][:], wts[bt][:, e:e + 1], acc[bt][:],
                                op0=ALU.mult, op1=ALU.add,
                            )

                # -- loss reduction
                ps_l = ps.tile([1, NBT], f32, tag="mm", name="ps_loss")
                nc.tensor.matmul(ps_l[:], ones_t[:, 0:1], lossv[:], start=True, stop=True)
                ls_sb = sp.tile([1, 1], f32, tag="lsum", name="ls_sb")
                nc.vector.tensor_reduce(ls_sb[:], ps_l[:], axis=AX.X, op=ALU.add)
                nc.sync.dma_start(out_loss[:], ls_sb[:])

                # -- tower
                tt = []
                for f in range(2):
                    t = sp.tile([128, BS], f32, tag=f"tt{f}", name=f"towerT{f}")
                    tt.append(t)
                for bt in range(NBT):
                    for f in range(2):
                        ptp = ps.tile([128, 128], f32, tag="mm", name=f"pst{bt}_{f}")
                        nc.tensor.transpose(ptp[:], acc[bt][:, f * 128:(f + 1) * 128], ident_t[:])
                        nc.scalar.copy(tt[f][:, bt * 128:(bt + 1) * 128], ptp[:])
                tts = [split_act(tt[f][:], hp, f"tts{f}", f"tts{f}") for f in range(2)]

                def twslice(w, c0, c1):
                    if x3:
                        return (w[0][:, c0:c1], w[1][:, c0:c1])
                    return w[:, c0:c1]

                h1s = []
                for f in range(2):
                    ph = ps.tile([128, BS], f32, tag="mm", name=f"ps_h1_{f}")
                    for k in range(2):
                        mm3(ph[:], twslice(tw1s, k * D + f * 128, k * D + f * 128 + 128), tts[k],
                            first=(k == 0), last=(k == 1))
                    t = fp.tile([128, BS], f32, tag="twf", name=f"h1_{f}", bufs=(3 if x3 else 4))
                    nc.scalar.activation(t[:], ph[:], AF.Relu, bias=tb1t[:, f:f + 1], scale=1.0)
                    h1s.append(split_act(t[:], hp, f"h1s{f}", f"h1s_{f}"))
                h2f = []
                for f in range(2):
                    ph = ps.tile([128, BS], f32, tag="mm", name=f"ps_h2_{f}")
                    for k in range(2):
                        mm3(ph[:], twslice(tw2s, k * D + f * 128, k * D + f * 128 + 128), h1s[k],
                            first=(k == 0), last=(k == 1))
                    t = fp.tile([128, BS], f32, tag="twf2", name=f"h2_{f}")
                    nc.scalar.activation(t[:], ph[:], AF.Relu, bias=tb2t[:, f:f + 1], scale=1.0)
                    h2f.append(t)
                ps_q = ps.tile([1, BS], f32, tag="mm", name="ps_q")
                for k in range(2):
                    nc.tensor.matmul(ps_q[:], tw3t[:, k:k + 1], h2f[k][:], start=(k == 0), stop=(k == 1))
                q_sb = sp.tile([1, BS], f32, tag="qsb", name="q_sb")
                nc.scalar.activation(q_sb[:], ps_q[:], AF.Identity, bias=tb3t[0:1, 0:1], scale=1.0)
                nc.sync.dma_start(out_q[:], q_sb[:])

            for _r in range(reps):
                emit_body(_r)

    split_multi_waits(nc)
    return nc


_NC_CACHE = {}


def _get_nc(mode=MODE):
    if mode not in _NC_CACHE:
        _NC_CACHE[mode] = build_nc(mode=mode)
    return _NC_CACHE[mode]


def prepare_in_maps(state_feat, act, task_id, rep_W1, rep_b1, rep_W2, rep_b2, emb,
                    kW1, kb1, kW2, kb2, vW1, vb1, vW2, vb2,
                    tW1, tb1, tW2, tb2, tW3, tb3, mode=MODE):
    x3 = mode == "f16x3"
    f32c = lambda a: np.ascontiguousarray(np.asarray(a, np.float32))

    x = np.concatenate([np.asarray(state_feat, np.float32),
                        np.asarray(act, np.float32)], axis=1)
    xp_ = np.zeros((B, XF), np.float32)
    xp_[:, :OBS + ACT] = x
    query = np.tanh(np.asarray(emb, np.float32))[np.asarray(task_id)]   # [B, D]
    qkb = query @ f32c(kb2).T                                           # [B, E]

    w1 = np.zeros((XF, H1), np.float32)
    w1[:OBS + ACT, :] = np.asarray(rep_W1, np.float32)
    w2 = f32c(rep_W2)
    kw1 = f32c(kW1).reshape(E * H1, H2)
    vw1 = f32c(vW1).reshape(E * H1, H2)
    kw2 = f32c(kW2).reshape(E * H2, D)
    vw2 = f32c(vW2).reshape(E * H2, D)

    def hl(a):
        hi = a.astype(np.float16)
        lo = (a - hi.astype(np.float32)).astype(np.float16)
        return np.ascontiguousarray(hi), np.ascontiguousarray(lo)

    shared = {}
    for nm, a in [("w1", w1), ("w2", w2), ("kw1", kw1), ("vw1", vw1),
                  ("kw2", kw2), ("vw2", vw2)]:
        if x3:
            shared[nm + "h"], shared[nm + "l"] = hl(a)
        else:
            shared[nm + "h"] = np.ascontiguousarray(a)

    shared.update({
        "b1h": np.ascontiguousarray(f32c(rep_b1).reshape(MK1, 128).T),
        "b2h": np.ascontiguousarray(f32c(rep_b2).reshape(MK1, 128).T),
        "kb1h": np.ascontiguousarray(f32c(kb1).reshape(E, MH, 128).transpose(2, 0, 1).reshape(128, E * MH)),
        "vb1h": np.ascontiguousarray(f32c(vb1).reshape(E, MH, 128).transpose(2, 0, 1).reshape(128, E * MH)),
        "vb2f": f32c(vb2).reshape(1, E * D),
        "tw1h": np.ascontiguousarray(f32c(tW1).reshape(2, 128, D).transpose(1, 0, 2).reshape(128, 2 * D)),
        "tw2h": np.ascontiguousarray(f32c(tW2).reshape(2, 128, D).transpose(1, 0, 2).reshape(128, 2 * D)),
        "tw3h": np.ascontiguousarray(f32c(tW3).reshape(2, 128).T),
        "tb1h": np.ascontiguousarray(f32c(tb1).reshape(2, 128).T),
        "tb2h": np.ascontiguousarray(f32c(tb2).reshape(2, 128).T),
        "tb3h": f32c(tb3).reshape(1, 1),
        "onesd": np.ones((128, 128), np.float32),
        "identd": np.eye(128, dtype=np.float32),
    })

    in_maps = []
    for c in range(NCORES):
        sl = slice(c * BS, (c + 1) * BS)
        m = dict(shared)
        xs = np.ascontiguousarray(xp_[sl].T)
        if x3:
            m["xTh"], m["xTl"] = hl(xs)
        else:
            m["xTh"] = xs
        m["qh"] = np.ascontiguousarray(query[sl])
        m["qkb"] = np.ascontiguousarray(qkb[sl])
        in_maps.append(m)
    return in_maps


def kernel(**inputs):
    in_maps = prepare_in_maps(**inputs)
    nc = _get_nc()
    res = run_bass_kernel_spmd(nc, in_maps, core_ids=list(range(NCORES)))
    q = np.concatenate([res.results[c]["out_q"][0] for c in range(NCORES)])
    total = sum(float(res.results[c]["out_loss"][0, 0]) for c in range(NCORES))
    expert_loss = np.float32(-0.3 * total / B)
    return q.astype(np.float32), expert_loss


# revision 15
# speedup vs baseline: 1.5493x; 1.4417x over previous
"""Trainium2 Bass kernel for MixExpertAttentionSACNetwork.

Data-parallel over batch: B=4096 split as 512 rows per core across 8 cores,
parameters replicated. Per-core dataflow:

  repT1 = relu(W1p.T @ xT + b1)          feat-on-partition [1024, 512b]
  repT2 = W2.T @ repT1 + b2              feat-on-partition [1024, 512b]
  keys pass (per expert e):
    hkT  = relu(kW1[e].T @ repT2 + kb1)  [512, 512b]
    keys = hkT.T-blocks @ kW2[e]         batch-on-partition [128b, 256] in PSUM
    scores[:, e] = rowsum(keys * q)      (DVE scalar_tensor_tensor, reads PSUM)
  scores += q @ kb2.T (host-folded offset); softmax; expert loss
  vals pass (per expert e):
    hvT, vals as above (+vb2 via ones-matmul); tower_in += w[:, e] * vals
  tower: transpose tower_in -> 3-layer MLP -> q [512]

Matmul precision modes:
  f16x3: operands split into fp16 hi+lo on host (weights) / device
         (activations); each product = 3 fp16 matmuls (hh, hl, lh) at
         1 cyc/row -> 3 cyc/row total, measured MORE accurate than native
         f32 matmul (which costs 4 cyc/row).
  f32:   native f32 matmuls.
  f32r:  reduced-precision 1 cyc/row mode (fast, ~1e-4 rel err).
"""
import sys
sys.path.insert(0, "/opt/trn_rl_repo")

import numpy as np
import concourse.bass as bass
import concourse.mybir as mybir
import concourse.tile as tile
from concourse.bass_utils import run_bass_kernel_spmd

f32 = mybir.dt.float32
f16 = mybir.dt.float16
AF = mybir.ActivationFunctionType
ALU = mybir.AluOpType
AX = mybir.AxisListType

B, OBS, ACT, NT, E, H1, H2, D = 4096, 390, 20, 10, 8, 1024, 512, 256
NCORES = 8
BS = B // NCORES            # 512 rows per core
NBT = BS // 128             # 4 batch tiles
XF = 512                    # padded input feature dim (410 -> 512)
MK1 = H1 // 128             # 8
KX = XF // 128              # 4
MH = H2 // 128              # 4

MODE = "f16x3"              # "f16x3" | "f32" | "f32r"


def split_multi_waits(nc, max_waits=1):
    """This walrus build accepts at most one semaphore wait per instruction;
    hoist extra waits onto single-wait NoOps on the same engine."""
    total = 0
    for fn in nc.m.functions:
        for blk in fn.blocks:
            new_insts = []
            for ins in blk.instructions:
                si = ins.sync_info
                waits = list(si.on_wait) if si and si.on_wait else []
                if len(waits) > max_waits:
                    total += 1
                    keep = waits[-max_waits:]
                    for i, w in enumerate(waits[:-max_waits]):
                        nop = mybir.InstNoOp(name=f"{ins.name}_w{i}", ins=[], outs=[])
                        nop.engine = ins.engine
                        nop.sync_info = mybir.SyncInfo(on_wait=[w], on_update=[])
                        new_insts.append(nop)
                    ins.sync_info = mybir.SyncInfo(
                        on_wait=keep, on_update=list(si.on_update or [])
                    )
                new_insts.append(ins)
            blk.instructions[:] = new_insts
    return total


def build_nc(mode=MODE, reps=1, trace_sim=False):
    x3 = mode == "f16x3"
    nc = bass.Bass(trn_type="TRN2")
    wdt = f16 if x3 else f32   # dram dtype for matmul weights

    def din(name, shape, dt=f32):
        return nc.dram_tensor(name, shape, dt, kind="ExternalInput")

    xT = din("xTh", [XF, BS], wdt)
    xTl = din("xTl", [XF, BS], wdt) if x3 else None
    qh = din("qh", [BS, D])
    qkb = din("qkb", [BS, E])            # host-folded q @ kb2.T score offset
    w1 = din("w1h", [XF, H1], wdt)
    w1l = din("w1l", [XF, H1], wdt) if x3 else None
    w2 = din("w2h", [H1, H1], wdt)
    w2l = din("w2l", [H1, H1], wdt) if x3 else None
    kw1 = din("kw1h", [E * H1, H2], wdt)
    kw1l = din("kw1l", [E * H1, H2], wdt) if x3 else None
    vw1 = din("vw1h", [E * H1, H2], wdt)
    vw1l = din("vw1l", [E * H1, H2], wdt) if x3 else None
    kw2 = din("kw2h", [E * H2, D], wdt)
    kw2l = din("kw2l", [E * H2, D], wdt) if x3 else None
    vw2 = din("vw2h", [E * H2, D], wdt)
    vw2l = din("vw2l", [E * H2, D], wdt) if x3 else None
    b1h = din("b1h", [128, MK1])
    b2h = din("b2h", [128, MK1])
    kb1h = din("kb1h", [128, E * MH])
    vb1h = din("vb1h", [128, E * MH])
    vb2f = din("vb2f", [1, E * D])
    tw1h = din("tw1h", [128, 2 * D])
    tw2h = din("tw2h", [128, 2 * D])
    tw3h = din("tw3h", [128, 2])
    tb1h = din("tb1h", [128, 2])
    tb2h = din("tb2h", [128, 2])
    tb3h = din("tb3h", [1, 1])
    onesd = din("onesd", [128, 128])
    identd = din("identd", [128, 128])

    out_q = nc.dram_tensor("out_q", [1, BS], f32, kind="ExternalOutput")
    out_loss = nc.dram_tensor("out_loss", [1, 1], f32, kind="ExternalOutput")

    use_f32r = mode == "f32r"
    f32r = mybir.dt.float32r

    def MM(out, lhsT, rhs, start, stop):
        if use_f32r:
            lhsT, rhs = lhsT.bitcast(f32r), rhs.bitcast(f32r)
        nc.tensor.matmul(out, lhsT, rhs, start=start, stop=stop)

    def mm3(out, lhs, rhs, first, last):
        """lhs/rhs: (hi, lo) AP pairs in f16x3 mode, plain APs otherwise."""
        if not x3:
            MM(out, lhs, rhs, first, last)
            return
        lh, ll = lhs
        rh, rl = rhs
        nc.tensor.matmul(out, lh, rh, start=first, stop=False)
        nc.tensor.matmul(out, lh, rl, start=False, stop=False)
        nc.tensor.matmul(out, ll, rh, start=False, stop=last)

    with tile.TileContext(nc, trace_sim=trace_sim) as tc:
        with (
            tc.tile_pool(name="const", bufs=1) as cp,
            tc.tile_pool(name="xq", bufs=1) as xp,
            tc.tile_pool(name="wk", bufs=8) as wp,
            tc.tile_pool(name="wk2", bufs=8) as wp2,
            tc.tile_pool(name="act", bufs=1) as ap,
            tc.tile_pool(name="fsc", bufs=3) as fp,
            tc.tile_pool(name="hkv", bufs=2) as hp,
            tc.tile_pool(name="small", bufs=1) as sp,
            tc.tile_pool(name="scr", bufs=4) as scp,
            tc.tile_pool(name="ps", bufs=8, space="PSUM") as ps,
        ):
            # ---- critical-path preloads first: PE needs xt + w1k pairs
            xt = []
            for k in range(KX):
                th = xp.tile([128, BS], wdt, tag=f"xt{k}", name=f"xt{k}")
                nc.sync.dma_start(th[:], xT[k * 128:(k + 1) * 128, :])
                if x3:
                    tl = xp.tile([128, BS], wdt, tag=f"xtl{k}", name=f"xtl{k}")
                    nc.sync.dma_start(tl[:], xTl[k * 128:(k + 1) * 128, :])
                    xt.append((th[:], tl[:]))
                else:
                    xt.append(th[:])
            w1ks = []
            for k in range(KX):
                th = wp.tile([128, H1], wdt, tag="wk", name=f"w1k{k}")
                nc.sync.dma_start(th[:], w1[k * 128:(k + 1) * 128, :])
                if x3:
                    tl = wp.tile([128, H1], wdt, tag="wkl", name=f"w1k{k}l")
                    nc.sync.dma_start(tl[:], w1l[k * 128:(k + 1) * 128, :])
                    w1ks.append((th[:], tl[:]))
                else:
                    w1ks.append(th[:])

            # ---- constants
            ones_t = cp.tile([128, 128], f32, tag="ones")
            nc.sync.dma_start(ones_t[:], onesd[:])
            ident_t = cp.tile([128, 128], f32, tag="ident")
            nc.sync.dma_start(ident_t[:], identd[:])
            b1t = cp.tile([128, MK1], f32, tag="b1")
            nc.sync.dma_start(b1t[:], b1h[:])
            b2t = cp.tile([128, MK1], f32, tag="b2")
            nc.sync.dma_start(b2t[:], b2h[:])
            kb1t = cp.tile([128, E * MH], f32, tag="kb1")
            nc.sync.dma_start(kb1t[:], kb1h[:])
            vb1t = cp.tile([128, E * MH], f32, tag="vb1")
            nc.sync.dma_start(vb1t[:], vb1h[:])
            vb2t = cp.tile([1, E * D], f32, tag="vb2")
            nc.sync.dma_start(vb2t[:], vb2f[:])
            tw1t = cp.tile([128, 2 * D], f32, tag="tw1")
            nc.sync.dma_start(tw1t[:], tw1h[:])
            tw2t = cp.tile([128, 2 * D], f32, tag="tw2")
            nc.sync.dma_start(tw2t[:], tw2h[:])
            tw3t = cp.tile([128, 2], f32, tag="tw3")
            nc.sync.dma_start(tw3t[:], tw3h[:])
            tb1t = cp.tile([128, 2], f32, tag="tb1")
            nc.sync.dma_start(tb1t[:], tb1h[:])
            tb2t = cp.tile([128, 2], f32, tag="tb2")
            nc.sync.dma_start(tb2t[:], tb2h[:])
            tb3t = cp.tile([1, 1], f32, tag="tb3")
            nc.sync.dma_start(tb3t[:], tb3h[:])
            eps10 = cp.tile([128, 1], f32, tag="eps10")
            nc.vector.memset(eps10[:], 1e-10)
            ones16 = cp.tile([1, 128], f16, tag="ones16")
            nc.vector.memset(ones16[:], 1.0)
            vb2h16 = cp.tile([1, E * D], f16, tag="vb2h16")
            nc.scalar.copy(vb2h16[:], vb2t[:])
            vb2l16 = cp.tile([1, E * D], f16, tag="vb2l16")
            nc.vector.tensor_sub(vb2l16[:], vb2t[:], vb2h16[:])

            # tower weight splits are invocation-invariant: do once
            if x3:
                tw1sh = cp.tile([128, 2 * D], f16, tag="tw1sh")
                nc.scalar.copy(tw1sh[:], tw1t[:])
                tw1sl = cp.tile([128, 2 * D], f16, tag="tw1sl")
                nc.vector.tensor_sub(tw1sl[:], tw1t[:], tw1sh[:])
                tw2sh = cp.tile([128, 2 * D], f16, tag="tw2sh")
                nc.scalar.copy(tw2sh[:], tw2t[:])
                tw2sl = cp.tile([128, 2 * D], f16, tag="tw2sl")
                nc.vector.tensor_sub(tw2sl[:], tw2t[:], tw2sh[:])
                tw1s = (tw1sh[:], tw1sl[:])
                tw2s = (tw2sh[:], tw2sl[:])
            else:
                tw1s, tw2s = tw1t[:], tw2t[:]

            def emit_body(rep):
                # -- per-invocation inputs (x preloaded at kernel head)
                qt, qkbt = [], []
                for bt in range(NBT):
                    t = xp.tile([128, D], f32, tag=f"qt{bt}", name=f"qt{bt}")
                    nc.sync.dma_start(t[:], qh[bt * 128:(bt + 1) * 128, :])
                    qt.append(t)
                    t2 = xp.tile([128, E], f32, tag=f"qkb{bt}", name=f"qkb{bt}")
                    nc.sync.dma_start(t2[:], qkb[bt * 128:(bt + 1) * 128, :])
                    qkbt.append(t2)

                def load_w(dram_h, dram_l, r0, r1, ncols, pool, tag, name):
                    th = pool.tile([128, ncols], wdt, tag=tag, name=name)
                    nc.sync.dma_start(th[:], dram_h[r0:r1, :])
                    if not x3:
                        return th[:]
                    tl = pool.tile([128, ncols], wdt, tag=tag + "l", name=name + "l")
                    nc.sync.dma_start(tl[:], dram_l[r0:r1, :])
                    return (th[:], tl[:])

                def wslice(w, c0, c1):
                    if x3:
                        return (w[0][:, c0:c1], w[1][:, c0:c1])
                    return w[:, c0:c1]

                def split_act(src_f32, pool, tagbase, name, bufs=None):
                    """f32 SBUF tile -> (hi, lo) fp16 pair in f16x3 mode."""
                    if not x3:
                        return src_f32
                    n = src_f32.shape[-1]
                    hi = pool.tile([128, n], f16, tag=tagbase + "h", name=name + "h", bufs=bufs)
                    nc.scalar.copy(hi[:], src_f32)
                    lo = pool.tile([128, n], f16, tag=tagbase + "l", name=name + "l", bufs=bufs)
                    nc.vector.tensor_sub(lo[:], src_f32, hi[:])
                    return (hi[:], lo[:])

                # -- repT1
                ps_r1 = [ps.tile([128, BS], f32, tag="mm", name=f"psr1_{m}") for m in range(MK1)]
                for k in range(KX):
                    w1k = w1ks[k]
                    for m in range(MK1):
                        mm3(ps_r1[m][:], wslice(w1k, m * 128, (m + 1) * 128), xt[k],
                            first=(k == 0), last=(k == KX - 1))
                rep1 = []
                for m in range(MK1):
                    t = fp.tile([128, BS], f32, tag="r1f", name=f"rep1_{m}", bufs=(3 if x3 else MK1 + 1))
                    nc.scalar.activation(t[:], ps_r1[m][:], AF.Relu, bias=b1t[:, m:m + 1], scale=1.0)
                    rep1.append(split_act(t[:], hp, "r1s", f"rep1s_{m}", bufs=MK1))

                # -- repT2 (splits persist through both expert passes -> act pool)
                ps_r2 = [ps.tile([128, BS], f32, tag="mm", name=f"psr2_{m}") for m in range(MK1)]
                for k in range(MK1):
                    w2k = load_w(w2, w2l, k * 128, (k + 1) * 128, H1, wp, "wk", f"w2k{k}")
                    for m in range(MK1):
                        mm3(ps_r2[m][:], wslice(w2k, m * 128, (m + 1) * 128), rep1[k],
                            first=(k == 0), last=(k == MK1 - 1))
                rep2 = []
                for m in range(MK1):
                    t = fp.tile([128, BS], f32, tag="r2f", name=f"rep2_{m}", bufs=(3 if x3 else MK1 + 1))
                    nc.scalar.activation(t[:], ps_r2[m][:], AF.Identity, bias=b2t[:, m:m + 1], scale=1.0)
                    if x3:
                        hi = ap.tile([128, BS], f16, tag=f"r2h{m}", name=f"rep2h_{m}")
                        nc.scalar.copy(hi[:], t[:])
                        lo = ap.tile([128, BS], f16, tag=f"r2l{m}", name=f"rep2l_{m}")
                        nc.vector.tensor_sub(lo[:], t[:], hi[:])
                        rep2.append((hi[:], lo[:]))
                    else:
                        rep2.append(t[:])

                scores = [sp.tile([128, E], f32, tag=f"sc{bt}", name=f"scores{bt}") for bt in range(NBT)]

                def expert_pass(e, ew1, ew1l, ew2, ew2l, eb1t, kind, final_stop=True):
                    ps_h = [ps.tile([128, BS], f32, tag="mm", name=f"ps{kind}h{e}_{m}") for m in range(MH)]
                    for k in range(MK1):
                        wkt = load_w(ew1, ew1l, e * H1 + k * 128, e * H1 + (k + 1) * 128,
                                     H2, wp, "wk", f"{kind}w1_{e}_{k}")
                        for m in range(MH):
                            mm3(ps_h[m][:], wslice(wkt, m * 128, (m + 1) * 128), rep2[k],
                                first=(k == 0), last=(k == MK1 - 1))
                    ht = []
                    for m in range(MH):
                        t = fp.tile([128, BS], f32, tag="hf", name=f"h{kind}{e}_{m}", bufs=(3 if x3 else 2 * MH))
                        nc.scalar.activation(t[:], ps_h[m][:], AF.Relu,
                                             bias=eb1t[:, e * MH + m:e * MH + m + 1], scale=1.0)
                        ht.append(split_act(t[:], hp, f"h{kind}{m}", f"h{kind}s{e}_{m}"))
                    w2_tiles = [
                        load_w(ew2, ew2l, e * H2 + m * 128, e * H2 + (m + 1) * 128,
                               D, wp2, "kv", f"{kind}w2_{e}_{m}")
                        for m in range(MH)
                    ]
                    ps_o = []
                    for bt in range(NBT):
                        po = ps.tile([128, D], f32, tag="mm", name=f"ps{kind}o{e}_{bt}")
                        for m in range(MH):
                            if x3:
                                hh, hl = ht[m]
                                lhs = (hh[:, bt * 128:(bt + 1) * 128], hl[:, bt * 128:(bt + 1) * 128])
                            else:
                                lhs = ht[m][:, bt * 128:(bt + 1) * 128]
                            mm3(po[:], lhs, w2_tiles[m], first=(m == 0),
                                last=(final_stop and m == MH - 1))
                        ps_o.append(po)
                    return ps_o

                # -- keys pass
                for e in range(E):
                    ps_o = expert_pass(e, kw1, kw1l, kw2, kw2l, kb1t, "k")
                    for bt in range(NBT):
                        scr = scp.tile([128, D], f32, tag="scr", name=f"scrk{e}_{bt}")
                        nc.vector.scalar_tensor_tensor(
                            scr[:], ps_o[bt][:], 1.0, qt[bt][:],
                            op0=ALU.mult, op1=ALU.mult,
                            accum_out=scores[bt][:, e:e + 1],
                        )

                # -- softmax + loss pieces (fold in host-computed q@kb2.T first)
                wts = []
                lossv = sp.tile([128, NBT], f32, tag="lossv", name="lossv")
                for bt in range(NBT):
                    nc.vector.tensor_add(scores[bt][:], scores[bt][:], qkbt[bt][:])
                    negmax = sp.tile([128, 1], f32, tag=f"nm{bt}", name=f"negmax{bt}")
                    nc.vector.tensor_reduce(negmax[:], scores[bt][:], axis=AX.X, op=ALU.max, negate=True)
                    expw = sp.tile([128, E], f32, tag=f"ew{bt}", name=f"expw{bt}")
                    nc.scalar.activation(expw[:], scores[bt][:], AF.Exp, bias=negmax[:, 0:1], scale=1.0)
                    ssum = sp.tile([128, 1], f32, tag=f"ss{bt}", name=f"ssum{bt}")
                    nc.vector.tensor_reduce(ssum[:], expw[:], axis=AX.X, op=ALU.add)
                    rinv = sp.tile([128, 1], f32, tag=f"ri{bt}", name=f"rinv{bt}")
                    nc.vector.reciprocal(rinv[:], ssum[:])
                    wt = sp.tile([128, E], f32, tag=f"wt{bt}", name=f"wt{bt}")
                    nc.vector.tensor_scalar_mul(wt[:], expw[:], rinv[:, 0:1])
                    wts.append(wt)
                    logw = sp.tile([128, E], f32, tag=f"lw{bt}", name=f"logw{bt}")
                    nc.scalar.activation(logw[:], wt[:], AF.Ln, bias=eps10[:, 0:1], scale=1.0)
                    clipw = sp.tile([128, E], f32, tag=f"cw{bt}", name=f"clipw{bt}")
                    nc.vector.tensor_scalar(clipw[:], logw[:], -6.0, 0.0, op0=ALU.max, op1=ALU.min)
                    nc.vector.tensor_reduce(lossv[:, bt:bt + 1], clipw[:], axis=AX.X, op=ALU.add)

                # -- vals pass (vb2 bias via ones-matmul, weighted accumulate)
                acc = [ap.tile([128, D], f32, tag=f"acc{bt}", name=f"acc{bt}") for bt in range(NBT)]
                for e in range(E):
                    ps_o = expert_pass(e, vw1, vw1l, vw2, vw2l, vb1t, "v", final_stop=False)
                    for bt in range(NBT):
                        nc.tensor.matmul(ps_o[bt][:], ones16[0:1, :], vb2h16[0:1, e * D:(e + 1) * D],
                                         start=False, stop=False, skip_group_check=True)
                        nc.tensor.matmul(ps_o[bt][:], ones16[0:1, :], vb2l16[0:1, e * D:(e + 1) * D],
                                         start=False, stop=True, skip_group_check=True)
                        if e == 0:
                            nc.vector.tensor_scalar_mul(acc[bt][:], ps_o[bt][:], wts[bt][:, 0:1])
                        else:
                            nc.vector.scalar_tensor_tensor(
                                acc[bt][:], ps_o[bt][:], wts[bt][:, e:e + 1], acc[bt][:],
                                op0=ALU.mult, op1=ALU.add,
                            )

                # -- loss reduction
                ps_l = ps.tile([1, NBT], f32, tag="mm", name="ps_loss")
                nc.tensor.matmul(ps_l[:], ones_t[:, 0:1], lossv[:], start=True, stop=True)
                ls_sb = sp.tile([1, 1], f32, tag="lsum", name="ls_sb")
                nc.vector.tensor_reduce(ls_sb[:], ps_l[:], axis=AX.X, op=ALU.add)
                nc.sync.dma_start(out_loss[:], ls_sb[:])

                # -- tower
                tt = []
                for f in range(2):
                    t = sp.tile([128, BS], f32, tag=f"tt{f}", name=f"towerT{f}")
                    tt.append(t)
                for bt in range(NBT):
                    for f in range(2):
                        ptp = ps.tile([128, 128], f32, tag="mm", name=f"pst{bt}_{f}")
                        nc.tensor.transpose(ptp[:], acc[bt][:, f * 128:(f + 1) * 128], ident_t[:])
                        nc.scalar.copy(tt[f][:, bt * 128:(bt + 1) * 128], ptp[:])
                tts = [split_act(tt[f][:], hp, f"tts{f}", f"tts{f}") for f in range(2)]

                def twslice(w, c0, c1):
                    if x3:
                        return (w[0][:, c0:c1], w[1][:, c0:c1])
                    return w[:, c0:c1]

                h1s = []
                for f in range(2):
                    ph = ps.tile([128, BS], f32, tag="mm", name=f"ps_h1_{f}")
                    for k in range(2):
                        mm3(ph[:], twslice(tw1s, k * D + f * 128, k * D + f * 128 + 128), tts[k],
                            first=(k == 0), last=(k == 1))
                    t = fp.tile([128, BS], f32, tag="twf", name=f"h1_{f}", bufs=(3 if x3 else 4))
                    nc.scalar.activation(t[:], ph[:], AF.Relu, bias=tb1t[:, f:f + 1], scale=1.0)
                    h1s.append(split_act(t[:], hp, f"h1s{f}", f"h1s_{f}"))
                h2f = []
                for f in range(2):
                    ph = ps.tile([128, BS], f32, tag="mm", name=f"ps_h2_{f}")
                    for k in range(2):
                        mm3(ph[:], twslice(tw2s, k * D + f * 128, k * D + f * 128 + 128), h1s[k],
                            first=(k == 0), last=(k == 1))
                    t = fp.tile([128, BS], f32, tag="twf2", name=f"h2_{f}")
                    nc.scalar.activation(t[:], ph[:], AF.Relu, bias=tb2t[:, f:f + 1], scale=1.0)
                    h2f.append(t)
                ps_q = ps.tile([1, BS], f32, tag="mm", name="ps_q")
                for k in range(2):
                    nc.tensor.matmul(ps_q[:], tw3t[:, k:k + 1], h2f[k][:], start=(k == 0), stop=(k == 1))
                q_sb = sp.tile([1, BS], f32, tag="qsb", name="q_sb")
                nc.scalar.activation(q_sb[:], ps_q[:], AF.Identity, bias=tb3t[0:1, 0:1], scale=1.0)
                nc.sync.dma_start(out_q[:], q_sb[:])

            for _r in range(reps):
                emit_body(_r)

    split_multi_waits(nc)
    return nc


_NC_CACHE = {}


def _get_nc(mode=MODE):
    if mode not in _NC_CACHE:
        _NC_CACHE[mode] = build_nc(mode=mode)
    return _NC_CACHE[mode]


def prepare_in_maps(state_feat, act, task_id, rep_W1, rep_b1, rep_W2, rep_b2, emb,
                    kW1, kb1, kW2, kb2, vW1, vb1, vW2, vb2,
                    tW1, tb1, tW2, tb2, tW3, tb3, mode=MODE):
    x3 = mode == "f16x3"
    f32c = lambda a: np.ascontiguousarray(np.asarray(a, np.float32))

    x = np.concatenate([np.asarray(state_feat, np.float32),
                        np.asarray(act, np.float32)], axis=1)
    xp_ = np.zeros((B, XF), np.float32)
    xp_[:, :OBS + ACT] = x
    query = np.tanh(np.asarray(emb, np.float32))[np.asarray(task_id)]   # [B, D]
    qkb = query @ f32c(kb2).T                                           # [B, E]

    w1 = np.zeros((XF, H1), np.float32)
    w1[:OBS + ACT, :] = np.asarray(rep_W1, np.float32)
    w2 = f32c(rep_W2)
    kw1 = f32c(kW1).reshape(E * H1, H2)
    vw1 = f32c(vW1).reshape(E * H1, H2)
    kw2 = f32c(kW2).reshape(E * H2, D)
    vw2 = f32c(vW2).reshape(E * H2, D)

    def hl(a):
        hi = a.astype(np.float16)
        lo = (a - hi.astype(np.float32)).astype(np.float16)
        return np.ascontiguousarray(hi), np.ascontiguousarray(lo)

    shared = {}
    for nm, a in [("w1", w1), ("w2", w2), ("kw1", kw1), ("vw1", vw1),
                  ("kw2", kw2), ("vw2", vw2)]:
        if x3:
            shared[nm + "h"], shared[nm + "l"] = hl(a)
        else:
            shared[nm + "h"] = np.ascontiguousarray(a)

    shared.update({
        "b1h": np.ascontiguousarray(f32c(rep_b1).reshape(MK1, 128).T),
        "b2h": np.ascontiguousarray(f32c(rep_b2).reshape(MK1, 128).T),
        "kb1h": np.ascontiguousarray(f32c(kb1).reshape(E, MH, 128).transpose(2, 0, 1).reshape(128, E * MH)),
        "vb1h": np.ascontiguousarray(f32c(vb1).reshape(E, MH, 128).transpose(2, 0, 1).reshape(128, E * MH)),
        "vb2f": f32c(vb2).reshape(1, E * D),
        "tw1h": np.ascontiguousarray(f32c(tW1).reshape(2, 128, D).transpose(1, 0, 2).reshape(128, 2 * D)),
        "tw2h": np.ascontiguousarray(f32c(tW2).reshape(2, 128, D).transpose(1, 0, 2).reshape(128, 2 * D)),
        "tw3h": np.ascontiguousarray(f32c(tW3).reshape(2, 128).T),
        "tb1h": np.ascontiguousarray(f32c(tb1).reshape(2, 128).T),
        "tb2h": np.ascontiguousarray(f32c(tb2).reshape(2, 128).T),
        "tb3h": f32c(tb3).reshape(1, 1),
        "onesd": np.ones((128, 128), np.float32),
        "identd": np.eye(128, dtype=np.float32),
    })

    in_maps = []
    for c in range(NCORES):
        sl = slice(c * BS, (c + 1) * BS)
        m = dict(shared)
        xs = np.ascontiguousarray(xp_[sl].T)
        if x3:
            m["xTh"], m["xTl"] = hl(xs)
        else:
            m["xTh"] = xs
        m["qh"] = np.ascontiguousarray(query[sl])
        m["qkb"] = np.ascontiguousarray(qkb[sl])
        in_maps.append(m)
    return in_maps


def kernel(**inputs):
    in_maps = prepare_in_maps(**inputs)
    nc = _get_nc()
    res = run_bass_kernel_spmd(nc, in_maps, core_ids=list(range(NCORES)))
    q = np.concatenate([res.results[c]["out_q"][0] for c in range(NCORES)])
    total = sum(float(res.results[c]["out_loss"][0, 0]) for c in range(NCORES))
    expert_loss = np.float32(-0.3 * total / B)
    return q.astype(np.float32), expert_loss
